# revision 15
# baseline (speedup 1.0000x reference)
"""Trainium2 Bass kernel for nn_Attention_62861141344964.

Full-input contract: kernel(**inputs) takes the unsharded inputs and returns
the full-shape output. Internally shards across 8 NeuronCores as
(batch, head-pair): core c handles batch c//4 and heads {2*(c%4), 2*(c%4)+1}.
Each core computes RMSNorm-folded QKV for its two heads, simT->exp->AV
attention, and a partial output projection y_part = w_out[:, heads] @ out
+ b_out/4 + x/4. The host sums the 4 partials per batch (the unshard of the
output-sum sharding) and concatenates batches.
"""

import sys
import os

sys.path.insert(0, "/opt/trn_rl_repo")

import numpy as np

HEADS = 8
DH = 64
DIM = 512
B = 2
HWS = 48
N = HWS * HWS  # 2304
KT = 4  # k-tiles of 128 over DIM
JT = 18  # j-tiles of 128 over N
NBLKS = [(0, 512), (512, 512), (1024, 512), (1536, 512), (2048, 256)]

_CACHE = {}


def _build_program(debug=False):
    import concourse.bass as bass  # noqa: F401
    import concourse.mybir as mybir
    import concourse.tile as tile
    from concourse import bacc

    f32 = mybir.dt.float32
    f32r = mybir.dt.float32r
    AF = mybir.ActivationFunctionType
    OP = mybir.AluOpType

    nc = bacc.Bacc("TRN2", target_bir_lowering=False, debug=False, num_devices=8)

    x4_d = nc.dram_tensor("x4", [DIM, N], f32r, kind="ExternalInput").ap()
    wqk_d = nc.dram_tensor("wqk", [DIM, 256], f32r, kind="ExternalInput").ap()
    wv_d = nc.dram_tensor("wv", [DIM, 128], f32r, kind="ExternalInput").ap()
    wp1_d = nc.dram_tensor("wp1", [128, DIM], f32r, kind="ExternalInput").ap()
    wp2_d = nc.dram_tensor("wp2", [128, DIM], f32r, kind="ExternalInput").ap()
    y_d = nc.dram_tensor("y", [DIM, N], f32, kind="ExternalOutput").ap()
    s_scratch = nc.dram_tensor("s_scratch", [1, N], f32).ap()
    r_scratch = nc.dram_tensor("r_scratch", [10, 512], f32).ap()
    dbg = {}
    if debug:
        for nm, shp in [("q2", [128, N]), ("k2", [128, N]), ("vT", [128, JT, 130]),
                        ("onA", [128, N]), ("onB", [128, N]), ("s_bc", [128, N]),
                        ("sq_bc", [128, N]), ("s_colT", [128, JT]), ("P0", [128, 3, 512])]:
            dbg[nm] = nc.dram_tensor("dbg_" + nm, shp, f32, kind="ExternalOutput").ap()

    with tile.TileContext(nc) as tc:
        big = tc.alloc_tile_pool(name="big", bufs=1)
        work = tc.alloc_tile_pool(name="work", bufs=2)
        pg = tc.alloc_tile_pool(name="pg", bufs=2, space="PSUM")
        pav = tc.alloc_tile_pool(name="pav", bufs=2, space="PSUM")
        setup = tc.alloc_tile_pool(name="setup", bufs=1)

        # ---------- load inputs ----------
        x4s = big.tile([128, KT, N], f32r)  # x/4, row (kt*128+p) -> [p, kt, n]
        nc.sync.dma_start(x4s[:], x4_d.rearrange("(a p) n -> p a n", p=128))
        wqk_s = big.tile([128, KT, 256], f32r)
        nc.sync.dma_start(wqk_s[:], wqk_d.rearrange("(a p) m -> p a m", p=128))
        wv_s = big.tile([128, KT, 128], f32r)
        nc.sync.dma_start(wv_s[:], wv_d.rearrange("(a p) m -> p a m", p=128))
        wp1_s = big.tile([128, DIM], f32r)
        nc.sync.dma_start(wp1_s[:], wp1_d)
        wp2_s = big.tile([128, DIM], f32r)
        nc.sync.dma_start(wp2_s[:], wp2_d)

        ones_col = big.tile([128, 1], f32r)
        nc.vector.memset(ones_col[:].bitcast(f32), 1.0)

        # ---------- norm scales ----------
        # sumsq over channels via PE (ones lhsT), then s = sqrt(32/sumsq4),
        # sq = s/8 = sqrt(0.5/sumsq4), where sumsq4 = sum((x/4)^2).
        xsq = setup.tile([128, KT, N], f32r)
        nc.vector.tensor_tensor(xsq[:], x4s[:], x4s[:], OP.mult)
        t_row = setup.tile([1, N], f32)
        for (o, w) in NBLKS:
            ps = pav.tile([1, 512], f32, tag="avy", name=f"ps_{o}")
            for kt in range(KT):
                nc.tensor.matmul(
                    ps[:, :w],
                    ones_col[:],
                    xsq[:, kt, o : o + w],
                    start=(kt == 0),
                    stop=(kt == KT - 1),
                )
            nc.vector.reciprocal(t_row[:, o : o + w], ps[:, :w])
        s_row = setup.tile([1, N], f32)
        sq_row = setup.tile([1, N], f32)
        nc.scalar.activation(s_row[:], t_row[:], AF.Sqrt, scale=32.0)
        nc.scalar.activation(sq_row[:], t_row[:], AF.Sqrt, scale=0.5)
        s_bc = big.tile([128, N], f32)
        nc.gpsimd.partition_broadcast(s_bc[:], s_row[:])
        sq_bc = big.tile([128, N], f32)
        nc.gpsimd.partition_broadcast(sq_bc[:], sq_row[:])
        s_colT = big.tile([128, JT], f32)  # s[n] at [n%128, n//128]
        nc.sync.dma_start(s_scratch, s_row[:])
        nc.sync.dma_start(s_colT[:], s_scratch.rearrange("a (f p) -> (a p) f", p=128))
        setup.release()
        pwav = tc.alloc_tile_pool(name="pwav", bufs=3)
        ywork = tc.alloc_tile_pool(name="ywork", bufs=2)

        # ---------- q, k (both heads, [128, N]) ----------
        q2 = big.tile([128, N], f32r)
        k2 = big.tile([128, N], f32r)
        for (o, w) in NBLKS:
            pq = pav.tile([128, 512], f32, tag="avy", name=f"pq_{o}")
            for kt in range(KT):
                nc.tensor.matmul(
                    pq[:, :w],
                    wqk_s[:, kt, 0:128],
                    x4s[:, kt, o : o + w],
                    start=(kt == 0),
                    stop=(kt == KT - 1),
                )
            nc.vector.tensor_tensor(q2[:, o : o + w], pq[:, :w], sq_bc[:, o : o + w], OP.mult)
            pk = pav.tile([128, 512], f32, tag="avy", name=f"pk_{o}")
            for kt in range(KT):
                nc.tensor.matmul(
                    pk[:, :w],
                    wqk_s[:, kt, 128:256],
                    x4s[:, kt, o : o + w],
                    start=(kt == 0),
                    stop=(kt == KT - 1),
                )
            nc.vector.tensor_tensor(k2[:, o : o + w], pk[:, :w], s_bc[:, o : o + w], OP.mult)

        # ---------- vT tiles [128, JT, 130]: [v_h0(64) | 1 | v_h1(64) | 1] ----------
        vT = big.tile([128, JT, 130], f32r)
        for jt in range(JT):
            pv = pav.tile([128, 512], f32, tag="avy", name=f"pv_{jt}")
            for kt in range(KT):
                nc.tensor.matmul(
                    pv[:, :128],
                    x4s[:, kt, jt * 128 : (jt + 1) * 128],
                    wv_s[:, kt, :],
                    start=(kt == 0),
                    stop=(kt == KT - 1),
                )
            nc.vector.tensor_scalar_mul(vT[:, jt, 0:64], pv[:, 0:64], s_colT[:, jt : jt + 1])
            nc.vector.tensor_scalar_mul(vT[:, jt, 65:129], pv[:, 64:128], s_colT[:, jt : jt + 1])
        nc.vector.memset(vT[:, :, 64:65].bitcast(f32), 1.0)
        nc.vector.memset(vT[:, :, 129:130].bitcast(f32), 1.0)

        # ---------- out_norm staging [128, N]: rows 0:64 head, 64 ones, 65: zero ----------
        onA = big.tile([128, N], f32r)
        onB = big.tile([128, N], f32r)
        nc.vector.memset(onA[64:128, :].bitcast(f32), 0.0)
        nc.vector.memset(onB[64:128, :].bitcast(f32), 0.0)
        nc.gpsimd.memset(onA[64:65, :].bitcast(f32), 1.0)

        # ---------- attention + projection per i-block ----------
        for ib, (o, w) in enumerate(NBLKS):
            av = [
                pav.tile([65, 512], f32, tag="avy", name=f"av0_{ib}"),
                pav.tile([65, 512], f32, tag="avy", name=f"av1_{ib}"),
            ]
            # 36 tiles: t = jt*2 + h; waves of 3 into [128, 3, 512] psum groups
            for wv_i in range(12):
                g = pg.tile([128, 3, 512], f32, tag="G", name=f"g_{ib}_{wv_i}")
                for slot in range(3):
                    t = wv_i * 3 + slot
                    jt, h = t // 2, t % 2
                    nc.tensor.matmul(
                        g[:, slot, :w],
                        k2[64 * h : 64 * (h + 1), jt * 128 : (jt + 1) * 128],
                        q2[64 * h : 64 * (h + 1), o : o + w],
                        start=True,
                        stop=True,
                    )
                p_sb = pwav.tile([128, 3, 512], f32r, tag="P", name=f"p_{ib}_{wv_i}")
                nc.scalar.activation(p_sb[:, :, :w], g[:, :, :w], AF.Exp)
                if debug and ib == 0 and wv_i == 0:
                    nc.sync.dma_start(dbg["P0"], p_sb[:].bitcast(f32))
                for slot in range(3):
                    t = wv_i * 3 + slot
                    jt, h = t // 2, t % 2
                    nc.tensor.matmul(
                        av[h][:, :w],
                        vT[:, jt, 65 * h : 65 * h + 65],
                        p_sb[:, slot, :w],
                        start=(jt == 0),
                        stop=(jt == JT - 1),
                        skip_group_check=True,
                    )
            # normalize: on[0:64] = av[0:64] * (1/den); den = av row 64
            for h, on in ((0, onA), (1, onB)):
                rcp65 = work.tile([65, 512], f32, tag="rcp")
                nc.vector.reciprocal(rcp65[64:65, :w], av[h][64:65, :w])
                nc.sync.dma_start(r_scratch[2 * ib + h, :w], rcp65[64:65, :w])
                rcp0 = work.tile([1, 512], f32, tag="rcp0")
                nc.sync.dma_start(rcp0[:, :w], r_scratch[2 * ib + h : 2 * ib + h + 1, :w])
                rb = work.tile([128, 512], f32, tag="rb")
                nc.gpsimd.partition_broadcast(rb[:, :w], rcp0[:, :w])
                nc.vector.tensor_tensor(
                    on[0:64, o : o + w], av[h][0:64, :w], rb[0:64, :w], OP.mult
                )
            # projection + residual for this i-block
            ysb = ywork.tile([128, KT, 512], f32, tag="y")
            for ot in range(KT):
                py = pav.tile([128, 512], f32, tag="avy", name=f"py_{ib}_{ot}")
                nc.tensor.matmul(
                    py[:, :w],
                    wp1_s[:, ot * 128 : (ot + 1) * 128],
                    onA[:, o : o + w],
                    start=True,
                    stop=False,
                )
                nc.tensor.matmul(
                    py[:, :w],
                    wp2_s[:, ot * 128 : (ot + 1) * 128],
                    onB[:, o : o + w],
                    start=False,
                    stop=True,
                )
                nc.vector.tensor_tensor(
                    ysb[:, ot, :w], py[:, :w], x4s[:, ot, o : o + w], OP.add
                )
            nc.sync.dma_start(
                y_d.rearrange("(a p) n -> p a n", p=128)[:, :, o : o + w],
                ysb[:, :, :w],
            )

        if debug:
            nc.sync.dma_start(dbg["q2"], q2[:].bitcast(f32))
            nc.sync.dma_start(dbg["k2"], k2[:].bitcast(f32))
            nc.sync.dma_start(dbg["vT"], vT[:].bitcast(f32))
            nc.sync.dma_start(dbg["onA"], onA[:].bitcast(f32))
            nc.sync.dma_start(dbg["onB"], onB[:].bitcast(f32))
            nc.sync.dma_start(dbg["s_bc"], s_bc[:])
            nc.sync.dma_start(dbg["sq_bc"], sq_bc[:])
            nc.sync.dma_start(dbg["s_colT"], s_colT[:])
        for pool in (ywork, pwav, pav, pg, work, big):
            pool.release()

    nc.compile()
    return nc


def _get_program():
    if "nc" not in _CACHE:
        _CACHE["nc"] = _build_program()
    return _CACHE["nc"]


def make_in_maps(x, g, w_qkv, w_out, b_out):
    """Build the per-core input dicts for the SPMD launch."""
    x = np.asarray(x, dtype=np.float32)
    g = np.asarray(g, dtype=np.float32).reshape(DIM)
    w_qkv = np.asarray(w_qkv, dtype=np.float32)
    w_out = np.asarray(w_out, dtype=np.float32)
    b_out = np.asarray(b_out, dtype=np.float32)

    in_maps = []
    for c in range(8):
        beta = c // 4
        h0 = 2 * (c % 4)
        h1 = h0 + 1
        x4 = (x[beta].reshape(DIM, N) / 4.0).astype(np.float32)
        # w_qkv rows: q block [0:512], k block [512:1024], v block [1024:1536]
        qr = np.r_[h0 * DH : (h0 + 1) * DH, h1 * DH : (h1 + 1) * DH]
        wq = w_qkv[qr]            # [128, DIM]
        wk = w_qkv[DIM + qr]      # [128, DIM]
        wvv = w_qkv[2 * DIM + qr]  # [128, DIM]
        gw = (g[None, :] * 4.0).astype(np.float32)
        wqk = np.concatenate([wq * gw, wk * gw], axis=0).T.copy()  # [DIM, 256]
        wvt = (wvv * gw).T.copy()  # [DIM, 128]
        wp1 = np.zeros((128, DIM), dtype=np.float32)
        wp1[0:DH] = w_out[:, h0 * DH : (h0 + 1) * DH].T
        wp1[DH] = b_out / 4.0
        wp2 = np.zeros((128, DIM), dtype=np.float32)
        wp2[0:DH] = w_out[:, h1 * DH : (h1 + 1) * DH].T
        in_maps.append(
            {
                "x4": np.ascontiguousarray(x4),
                "wqk": np.ascontiguousarray(wqk),
                "wv": np.ascontiguousarray(wvt),
                "wp1": wp1,
                "wp2": wp2,
            }
        )
    return in_maps


def run_spmd(in_maps, trace=False):
    from concourse.bass_utils import run_bass_kernel_spmd

    nc = _get_program()
    return run_bass_kernel_spmd(nc, in_maps, list(range(8)), trace=trace)


def combine(results, x):
    x = np.asarray(x, dtype=np.float32)
    y = np.zeros((B, DIM, N), dtype=np.float32)
    for c in range(8):
        y[c // 4] += results[c]["y"]
    return y.reshape(B, DIM, HWS, HWS)


def kernel(x, g, w_qkv, w_out, b_out):
    in_maps = make_in_maps(x, g, w_qkv, w_out, b_out)
    res = run_spmd(in_maps)
    return combine(res.results, x)


# revision 29
# speedup vs baseline: 10193.8411x; 10193.8411x over previous
"""Trainium2 Bass kernel for nn_Attention_62861141344964.

Full-input contract: kernel(**inputs) takes the unsharded inputs and returns
the full-shape output. Internally shards across 8 NeuronCores as
(batch, head-pair): core c handles batch c//4 and heads {2*(c%4), 2*(c%4)+1}.
Each core computes RMSNorm-folded QKV for its two heads, simT->exp->AV
attention, and a partial output projection y_part = w_out[:, heads] @ out
+ b_out/4 + x/4. The host sums the 4 partials per batch (the unshard of the
output-sum sharding) and concatenates batches.
"""

import sys
import os

sys.path.insert(0, "/opt/trn_rl_repo")

import numpy as np

HEADS = 8
DH = 64
DIM = 512
B = 2
HWS = 48
N = HWS * HWS  # 2304
KT = 4  # k-tiles of 128 over DIM
JT = 18  # j-tiles of 128 over N
NBLKS = [(0, 512), (512, 512), (1024, 512), (1536, 512), (2048, 256)]

_CACHE = {}


def _build_program(debug=False):
    import concourse.bass as bass  # noqa: F401
    import concourse.mybir as mybir
    import concourse.tile as tile
    from concourse import bacc

    f32 = mybir.dt.float32
    f32r = mybir.dt.float32r
    AF = mybir.ActivationFunctionType
    OP = mybir.AluOpType

    nc = bacc.Bacc("TRN2", target_bir_lowering=False, debug=False, num_devices=8)

    x4_d = nc.dram_tensor("x4", [DIM, N], f32r, kind="ExternalInput").ap()
    wqk_d = nc.dram_tensor("wqk", [DIM, 256], f32r, kind="ExternalInput").ap()
    wv_d = nc.dram_tensor("wv", [DIM, 128], f32r, kind="ExternalInput").ap()
    wp1_d = nc.dram_tensor("wp1", [128, DIM], f32r, kind="ExternalInput").ap()
    wp2_d = nc.dram_tensor("wp2", [128, DIM], f32r, kind="ExternalInput").ap()
    y_d = nc.dram_tensor("y", [DIM, N], f32, kind="ExternalOutput").ap()
    s_scratch = nc.dram_tensor("s_scratch", [1, N], f32).ap()
    dbg = {}
    if debug:
        for nm, shp in [("q2", [128, N]), ("k2", [128, N]), ("vT", [128, JT, 130]),
                        ("onA", [128, N]), ("onB", [128, N]), ("s_bc", [128, N]),
                        ("sq_bc", [128, N]), ("s_colT", [128, JT]), ("P0", [128, 3, 512])]:
            dbg[nm] = nc.dram_tensor("dbg_" + nm, shp, f32, kind="ExternalOutput").ap()

    with tile.TileContext(nc) as tc:
        big = tc.alloc_tile_pool(name="big", bufs=1)
        work = tc.alloc_tile_pool(name="work", bufs=2)
        pg = tc.alloc_tile_pool(name="pg", bufs=1, space="PSUM")
        pav = tc.alloc_tile_pool(name="pav", bufs=3, space="PSUM")
        setup = tc.alloc_tile_pool(name="setup", bufs=1)

        # ---------- load inputs ----------
        wqk_s = big.tile([128, KT, 256], f32r)
        nc.sync.dma_start(wqk_s[:], wqk_d.rearrange("(a p) m -> p a m", p=128))
        wv_s = big.tile([128, KT, 128], f32r)
        nc.sync.dma_start(wv_s[:], wv_d.rearrange("(a p) m -> p a m", p=128))
        wp1_s = big.tile([128, DIM], f32r)
        nc.sync.dma_start(wp1_s[:], wp1_d)
        wp2_s = big.tile([128, DIM], f32r)
        nc.sync.dma_start(wp2_s[:], wp2_d)

        ones_col = big.tile([128, 1], f32r)
        nc.vector.memset(ones_col[:].bitcast(f32), 1.0)
        e64 = big.tile([128, 128], f32r)  # row 64 = ones: PE partition-bcast of row 64
        nc.vector.memset(e64[:].bitcast(f32), 0.0)
        nc.vector.memset(e64[64:65, :].bitcast(f32), 1.0)

        # ---------- pipelined prologue: per n-block load -> square -> sumsq ->
        # scales -> k/q projection, so PE/ACT/DVE ramp together ----------
        x4s = big.tile([128, KT, N], f32r)
        x4_r = x4_d.rearrange("(a p) n -> p a n", p=128)
        xsq = setup.tile([128, KT, N], f32r)
        t_row = setup.tile([1, N], f32)
        s_row = setup.tile([1, N], f32)
        sq_row = setup.tile([1, N], f32)
        s_bc = big.tile([128, N], f32)
        sq_bc = big.tile([128, N], f32)
        q2 = big.tile([128, N], f32r)
        k2 = big.tile([128, N], f32r)
        for (o, w) in NBLKS:
            nc.sync.dma_start(x4s[:, :, o : o + w], x4_r[:, :, o : o + w])
            nc.vector.tensor_tensor(
                xsq[:, :, o : o + w], x4s[:, :, o : o + w], x4s[:, :, o : o + w],
                OP.mult,
            )
            ps = pav.tile([1, 512], f32, tag="avy", name=f"ps_{o}")
            for kt in range(KT):
                nc.tensor.matmul(
                    ps[:, :w],
                    ones_col[:],
                    xsq[:, kt, o : o + w],
                    start=(kt == 0),
                    stop=(kt == KT - 1),
                )
            nc.vector.reciprocal(t_row[:, o : o + w], ps[:, :w])
            nc.scalar.activation(s_row[:, o : o + w], t_row[:, o : o + w], AF.Sqrt, scale=32.0)
            nc.scalar.activation(sq_row[:, o : o + w], t_row[:, o : o + w], AF.Sqrt, scale=0.5)
            nc.gpsimd.partition_broadcast(s_bc[:, o : o + w], s_row[:, o : o + w])
            nc.gpsimd.partition_broadcast(sq_bc[:, o : o + w], sq_row[:, o : o + w])
            pk = pav.tile([128, 512], f32, tag="avy", name=f"pk_{o}")
            for kt in range(KT):
                nc.tensor.matmul(
                    pk[:, :w],
                    wqk_s[:, kt, 128:256],
                    x4s[:, kt, o : o + w],
                    start=(kt == 0),
                    stop=(kt == KT - 1),
                )
            nc.vector.tensor_tensor(k2[:, o : o + w], pk[:, :w], s_bc[:, o : o + w], OP.mult)
            pq = pav.tile([128, 512], f32, tag="avy", name=f"pq_{o}")
            for kt in range(KT):
                nc.tensor.matmul(
                    pq[:, :w],
                    wqk_s[:, kt, 0:128],
                    x4s[:, kt, o : o + w],
                    start=(kt == 0),
                    stop=(kt == KT - 1),
                )
            nc.vector.tensor_tensor(q2[:, o : o + w], pq[:, :w], sq_bc[:, o : o + w], OP.mult)
        s_colT = big.tile([128, JT], f32)  # s[n] at [n%128, n//128]
        nc.sync.dma_start(s_scratch, s_row[:])
        nc.sync.dma_start(s_colT[:], s_scratch.rearrange("a (f p) -> (a p) f", p=128))
        setup.release()
        pwav = tc.alloc_tile_pool(name="pwav", bufs=3)
        ywork = tc.alloc_tile_pool(name="ywork", bufs=2)

        # ---------- vT tiles [128, JT, 130]: [v_h0(64) | 1 | v_h1(64) | 1] ----
        # Emitted lazily inside the attention loop (2 jobs per wave) so this
        # PE-only work hides under the ACT-bound exp stream.
        vT = big.tile([128, JT, 130], f32r)
        nc.vector.memset(vT[:, :, 64:65].bitcast(f32), 1.0)
        nc.vector.memset(vT[:, :, 129:130].bitcast(f32), 1.0)

        def vt_job(jt):
            def job():
                pv = pav.tile([128, 512], f32, tag="avy", name=f"pv_{jt}")
                for kt in range(KT):
                    nc.tensor.matmul(
                        pv[:, :128],
                        x4s[:, kt, jt * 128 : (jt + 1) * 128],
                        wv_s[:, kt, :],
                        start=(kt == 0),
                        stop=(kt == KT - 1),
                    )
                nc.vector.tensor_scalar_mul(
                    vT[:, jt, 0:64], pv[:, 0:64], s_colT[:, jt : jt + 1]
                )
                nc.vector.tensor_scalar_mul(
                    vT[:, jt, 65:129], pv[:, 64:128], s_colT[:, jt : jt + 1]
                )
            return job

        vt_jobs = [vt_job(jt) for jt in range(JT)]

        # ---------- out_norm staging [128, N]: rows 0:64 head, 64 ones, 65: zero ----------
        onA = big.tile([128, N], f32r)
        onB = big.tile([128, N], f32r)
        nc.gpsimd.memset(onA[64:128, :].bitcast(f32), 0.0)
        nc.gpsimd.memset(onB[64:128, :].bitcast(f32), 0.0)
        nc.gpsimd.memset(onA[64:65, :].bitcast(f32), 1.0)
        den_pad = [big.tile([128, 512], f32r, name="den_pad0"),
                   big.tile([128, 512], f32r, name="den_pad1")]
        nc.gpsimd.memset(den_pad[0][:].bitcast(f32), 0.0)
        nc.gpsimd.memset(den_pad[1][:].bitcast(f32), 0.0)

        # ---------- attention + projection per i-block ----------
        # 36 (jt, h) tiles per i-block; wave sizes alternate 3/2 so the two
        # psum groups (G3: 3 banks, G2: 2 banks) double-buffer; the finished
        # i-block's tail (normalize / project / store) is emitted inside the
        # next i-block's first waves so the PE queue never blocks on it.
        WAVE_SIZES = [3, 3, 2, 3, 2, 3, 2, 3, 2, 3, 2, 3, 2, 3]  # sum = 36

        def make_tail_norm(ib, o, w, av):
            def tail():
                for h, on in ((0, onA), (1, onB)):
                    nc.vector.tensor_copy(den_pad[h][64:65, :w], av[h][64:65, :w])
                    dbc = pav.tile([128, 512], f32, tag="avy", name=f"dbc_{ib}_{h}")
                    nc.tensor.matmul(
                        dbc[:, :w], e64[:], den_pad[h][:, :w],
                        start=True, stop=True,
                    )
                    rb = work.tile([128, 512], f32, tag="rb")
                    nc.vector.reciprocal(rb[:, :w], dbc[:, :w])
                    nc.vector.tensor_tensor(
                        on[0:64, o : o + w], av[h][0:64, :w], rb[0:64, :w], OP.mult
                    )
            return tail

        def make_tail_proj(ib, o, w):
            def tail():
                ysb = ywork.tile([128, KT, 512], f32, tag="y")
                for ot in range(KT):
                    py = pav.tile([128, 512], f32, tag="avy", name=f"py_{ib}_{ot}")
                    nc.tensor.matmul(
                        py[:, :w],
                        wp1_s[:, ot * 128 : (ot + 1) * 128],
                        onA[:, o : o + w],
                        start=True,
                        stop=False,
                    )
                    nc.tensor.matmul(
                        py[:, :w],
                        wp2_s[:, ot * 128 : (ot + 1) * 128],
                        onB[:, o : o + w],
                        start=False,
                        stop=True,
                    )
                    nc.vector.tensor_tensor(
                        ysb[:, ot, :w], py[:, :w], x4s[:, ot, o : o + w], OP.add
                    )
                nc.sync.dma_start(
                    y_d.rearrange("(a p) n -> p a n", p=128)[:, :, o : o + w],
                    ysb[:, :, :w],
                )
            return tail

        deferred = []
        for ib, (o, w) in enumerate(NBLKS):
            av = [
                pav.tile([65, 512], f32, tag="avy", name=f"av0_{ib}"),
                pav.tile([65, 512], f32, tag="avy", name=f"av1_{ib}"),
            ]
            pending = None
            t = 0
            for wv_i, sz in enumerate(WAVE_SIZES):
                tag = "G3" if sz == 3 else "G2"
                g = pg.tile([128, sz, 512], f32, tag=tag, name=f"g_{ib}_{wv_i}")
                tiles = []
                for slot in range(sz):
                    jt, h = t // 2, t % 2
                    t += 1
                    tiles.append((slot, jt, h))
                    nc.tensor.matmul(
                        g[:, slot, :w],
                        k2[64 * h : 64 * (h + 1), jt * 128 : (jt + 1) * 128],
                        q2[64 * h : 64 * (h + 1), o : o + w],
                        start=True,
                        stop=True,
                    )
                p_sb = pwav.tile([128, sz, 512], f32r, tag="P", name=f"p_{ib}_{wv_i}")
                nc.scalar.activation(p_sb[:, :, :w], g[:, :, :w], AF.Exp)
                if debug and ib == 0 and wv_i == 0:
                    nc.sync.dma_start(dbg["P0"], p_sb[:, :, :].bitcast(f32))
                if deferred and wv_i == 0:
                    deferred.pop(0)()
                for _ in range(2):
                    if vt_jobs:
                        vt_jobs.pop(0)()
                waves = [pending, (tiles, p_sb)] if pending else [(tiles, p_sb)]
                if wv_i < len(WAVE_SIZES) - 1:
                    pending = waves.pop()
                for tiles_j, psb_j in waves:
                    for slot, jt, h in tiles_j:
                        nc.tensor.matmul(
                            av[h][:, :w],
                            vT[:, jt, 65 * h : 65 * h + 65],
                            psb_j[:, slot, :w],
                            start=(jt == 0),
                            stop=(jt == JT - 1),
                            skip_group_check=True,
                        )
            make_tail_norm(ib, o, w, av)()
            deferred = [make_tail_proj(ib, o, w)]
        for fn in deferred:
            fn()

        if debug:
            nc.sync.dma_start(dbg["q2"], q2[:].bitcast(f32))
            nc.sync.dma_start(dbg["k2"], k2[:].bitcast(f32))
            nc.sync.dma_start(dbg["vT"], vT[:].bitcast(f32))
            nc.sync.dma_start(dbg["onA"], onA[:].bitcast(f32))
            nc.sync.dma_start(dbg["onB"], onB[:].bitcast(f32))
            nc.sync.dma_start(dbg["s_bc"], s_bc[:])
            nc.sync.dma_start(dbg["sq_bc"], sq_bc[:])
            nc.sync.dma_start(dbg["s_colT"], s_colT[:])
        for pool in (ywork, pwav, pav, pg, work, big):
            pool.release()

    nc.compile()
    return nc


def _get_program():
    if "nc" not in _CACHE:
        _CACHE["nc"] = _build_program()
    return _CACHE["nc"]


def make_in_maps(x, g, w_qkv, w_out, b_out):
    """Build the per-core input dicts for the SPMD launch."""
    x = np.asarray(x, dtype=np.float32)
    g = np.asarray(g, dtype=np.float32).reshape(DIM)
    w_qkv = np.asarray(w_qkv, dtype=np.float32)
    w_out = np.asarray(w_out, dtype=np.float32)
    b_out = np.asarray(b_out, dtype=np.float32)

    in_maps = []
    for c in range(8):
        beta = c // 4
        h0 = 2 * (c % 4)
        h1 = h0 + 1
        x4 = (x[beta].reshape(DIM, N) / 4.0).astype(np.float32)
        # w_qkv rows: q block [0:512], k block [512:1024], v block [1024:1536]
        qr = np.r_[h0 * DH : (h0 + 1) * DH, h1 * DH : (h1 + 1) * DH]
        wq = w_qkv[qr]            # [128, DIM]
        wk = w_qkv[DIM + qr]      # [128, DIM]
        wvv = w_qkv[2 * DIM + qr]  # [128, DIM]
        gw = (g[None, :] * 4.0).astype(np.float32)
        wqk = np.concatenate([wq * gw, wk * gw], axis=0).T.copy()  # [DIM, 256]
        wvt = (wvv * gw).T.copy()  # [DIM, 128]
        wp1 = np.zeros((128, DIM), dtype=np.float32)
        wp1[0:DH] = w_out[:, h0 * DH : (h0 + 1) * DH].T
        wp1[DH] = b_out / 4.0
        wp2 = np.zeros((128, DIM), dtype=np.float32)
        wp2[0:DH] = w_out[:, h1 * DH : (h1 + 1) * DH].T
        in_maps.append(
            {
                "x4": np.ascontiguousarray(x4),
                "wqk": np.ascontiguousarray(wqk),
                "wv": np.ascontiguousarray(wvt),
                "wp1": wp1,
                "wp2": wp2,
            }
        )
    return in_maps


def run_spmd(in_maps, trace=False):
    from concourse.bass_utils import run_bass_kernel_spmd

    nc = _get_program()
    return run_bass_kernel_spmd(nc, in_maps, list(range(8)), trace=trace)


def combine(results, x):
    x = np.asarray(x, dtype=np.float32)
    y = np.zeros((B, DIM, N), dtype=np.float32)
    for c in range(8):
        y[c // 4] += results[c]["y"]
    return y.reshape(B, DIM, HWS, HWS)


def kernel(x, g, w_qkv, w_out, b_out):
    in_maps = make_in_maps(x, g, w_qkv, w_out, b_out)
    res = run_spmd(in_maps)
    return combine(res.results, x)


# revision 45
# speedup vs baseline: 10755.4410x; 1.0551x over previous
"""Trainium2 Bass kernel for nn_Attention_62861141344964.

Full-input contract: kernel(**inputs) takes the unsharded inputs and returns
the full-shape output. Internally shards across 8 NeuronCores as
(batch, head-pair): core c handles batch c//4 and heads {2*(c%4), 2*(c%4)+1}.
Each core computes RMSNorm-folded QKV for its two heads, simT->exp->AV
attention, and a partial output projection y_part = w_out[:, heads] @ out
+ b_out/4 + x/4. The host sums the 4 partials per batch (the unshard of the
output-sum sharding) and concatenates batches.
"""

import sys
import os

sys.path.insert(0, "/opt/trn_rl_repo")

import numpy as np

HEADS = 8
DH = 64
DIM = 512
B = 2
HWS = 48
N = HWS * HWS  # 2304
KT = 4  # k-tiles of 128 over DIM
JT = 18  # j-tiles of 128 over N
NBLKS = [(0, 512), (512, 512), (1024, 512), (1536, 512), (2048, 256)]

_CACHE = {}


def _build_program(debug=False):
    import concourse.bass as bass  # noqa: F401
    import concourse.mybir as mybir
    import concourse.tile as tile
    from concourse import bacc

    f32 = mybir.dt.float32
    f32r = mybir.dt.float32r
    AF = mybir.ActivationFunctionType
    OP = mybir.AluOpType

    nc = bacc.Bacc("TRN2", target_bir_lowering=False, debug=False, num_devices=8)

    x4_d = nc.dram_tensor("x4", [DIM, N], f32r, kind="ExternalInput").ap()
    wqk_d = nc.dram_tensor("wqk", [DIM, 256], f32r, kind="ExternalInput").ap()
    wv_d = nc.dram_tensor("wv", [DIM, 128], f32r, kind="ExternalInput").ap()
    wp1_d = nc.dram_tensor("wp1", [128, DIM], f32r, kind="ExternalInput").ap()
    wp2_d = nc.dram_tensor("wp2", [128, DIM], f32r, kind="ExternalInput").ap()
    y_d = nc.dram_tensor("y", [DIM, N], f32, kind="ExternalOutput").ap()
    s_scratch = nc.dram_tensor("s_scratch", [1, N], f32).ap()
    dbg = {}
    if debug:
        for nm, shp in [("q2", [128, N]), ("k2", [128, N]), ("vT", [128, JT, 130]),
                        ("onA", [128, N]), ("onB", [128, N]), ("s_bc", [128, N]),
                        ("sq_bc", [128, N]), ("s_colT", [128, JT]), ("P0", [128, 3, 512])]:
            dbg[nm] = nc.dram_tensor("dbg_" + nm, shp, f32, kind="ExternalOutput").ap()

    with tile.TileContext(nc) as tc:
        big = tc.alloc_tile_pool(name="big", bufs=1)
        work = tc.alloc_tile_pool(name="work", bufs=2)
        pg = tc.alloc_tile_pool(name="pg", bufs=1, space="PSUM")
        pav = tc.alloc_tile_pool(name="pav", bufs=3, space="PSUM")
        setup = tc.alloc_tile_pool(name="setup", bufs=1)

        # ---------- load inputs ----------
        # weights ride the ACT HWDGE ring so the x4 loads (SP ring) are not
        # queued behind them at startup
        wqk_s = big.tile([128, KT, 256], f32r)
        nc.scalar.dma_start(wqk_s[:], wqk_d.rearrange("(a p) m -> p a m", p=128))
        wv_s = big.tile([128, KT, 128], f32r)
        nc.scalar.dma_start(wv_s[:], wv_d.rearrange("(a p) m -> p a m", p=128))
        wp1_s = big.tile([128, DIM], f32r)
        nc.scalar.dma_start(wp1_s[:], wp1_d)
        wp2_s = big.tile([128, DIM], f32r)
        nc.scalar.dma_start(wp2_s[:], wp2_d)

        ones_col = big.tile([128, 1], f32r)
        nc.vector.memset(ones_col[:].bitcast(f32), 1.0)
        e64 = big.tile([128, 128], f32r)  # row 64 = ones: PE partition-bcast of row 64
        nc.vector.memset(e64[:].bitcast(f32), 0.0)
        nc.vector.memset(e64[64:65, :].bitcast(f32), 1.0)

        # ---------- pipelined prologue: per n-block load -> square -> sumsq ->
        # scales -> k/q projection, so PE/ACT/DVE ramp together ----------
        x4s = big.tile([128, KT, N], f32r)
        x4_r = x4_d.rearrange("(a p) n -> p a n", p=128)
        xsq = setup.tile([128, KT, N], f32r)
        t_row = setup.tile([1, N], f32)
        s_row = setup.tile([1, N], f32)
        sq_row = setup.tile([1, N], f32)
        s_bc = big.tile([128, N], f32)
        sq_bc = big.tile([128, N], f32)
        q2 = big.tile([128, N], f32r)
        k2 = big.tile([128, N], f32r)
        for (o, w) in NBLKS:
            nc.sync.dma_start(x4s[:, :, o : o + w], x4_r[:, :, o : o + w])
            nc.vector.tensor_tensor(
                xsq[:, :, o : o + w], x4s[:, :, o : o + w], x4s[:, :, o : o + w],
                OP.mult,
            )
            ps = pav.tile([1, 512], f32, tag="avy", name=f"ps_{o}")
            for kt in range(KT):
                nc.tensor.matmul(
                    ps[:, :w],
                    ones_col[:],
                    xsq[:, kt, o : o + w],
                    start=(kt == 0),
                    stop=(kt == KT - 1),
                )
            nc.vector.reciprocal(t_row[:, o : o + w], ps[:, :w])
            nc.scalar.activation(s_row[:, o : o + w], t_row[:, o : o + w], AF.Sqrt, scale=32.0)
            nc.scalar.activation(sq_row[:, o : o + w], t_row[:, o : o + w], AF.Sqrt, scale=0.5)
            nc.gpsimd.partition_broadcast(s_bc[:, o : o + w], s_row[:, o : o + w])
            nc.gpsimd.partition_broadcast(sq_bc[:, o : o + w], sq_row[:, o : o + w])
            pk = pav.tile([128, 512], f32, tag="avy", name=f"pk_{o}")
            for kt in range(KT):
                nc.tensor.matmul(
                    pk[:, :w],
                    wqk_s[:, kt, 128:256],
                    x4s[:, kt, o : o + w],
                    start=(kt == 0),
                    stop=(kt == KT - 1),
                )
            nc.vector.tensor_tensor(k2[:, o : o + w], pk[:, :w], s_bc[:, o : o + w], OP.mult)
            pq = pav.tile([128, 512], f32, tag="avy", name=f"pq_{o}")
            for kt in range(KT):
                nc.tensor.matmul(
                    pq[:, :w],
                    wqk_s[:, kt, 0:128],
                    x4s[:, kt, o : o + w],
                    start=(kt == 0),
                    stop=(kt == KT - 1),
                )
            nc.vector.tensor_tensor(q2[:, o : o + w], pq[:, :w], sq_bc[:, o : o + w], OP.mult)
        s_colT = big.tile([128, JT], f32)  # s[n] at [n%128, n//128]
        nc.sync.dma_start(s_scratch, s_row[:])
        nc.sync.dma_start(s_colT[:], s_scratch.rearrange("a (f p) -> (a p) f", p=128))
        setup.release()
        pwav = tc.alloc_tile_pool(name="pwav", bufs=6)
        ywork = tc.alloc_tile_pool(name="ywork", bufs=3)

        # ---------- vT tiles [128, JT, 130]: [v_h0(64) | 1 | v_h1(64) | 1] ----
        # Emitted lazily inside the attention loop (2 jobs per wave) so this
        # PE-only work hides under the ACT-bound exp stream.
        vT = big.tile([128, JT, 130], f32r)
        nc.vector.memset(vT[:, :, 64:65].bitcast(f32), 1.0)
        nc.vector.memset(vT[:, :, 129:130].bitcast(f32), 1.0)

        def vt_job(jt):
            def job():
                pv = pav.tile([128, 512], f32, tag="avy", name=f"pv_{jt}")
                for kt in range(KT):
                    nc.tensor.matmul(
                        pv[:, :128],
                        x4s[:, kt, jt * 128 : (jt + 1) * 128],
                        wv_s[:, kt, :],
                        start=(kt == 0),
                        stop=(kt == KT - 1),
                    )
                nc.vector.tensor_scalar_mul(
                    vT[:, jt, 0:64], pv[:, 0:64], s_colT[:, jt : jt + 1]
                )
                nc.vector.tensor_scalar_mul(
                    vT[:, jt, 65:129], pv[:, 64:128], s_colT[:, jt : jt + 1]
                )
            return job

        vt_jobs = [vt_job(jt) for jt in range(JT)]

        # ---------- out_norm staging [128, N]: rows 0:64 head, 64 ones, 65: zero ----------
        onA = big.tile([128, N], f32r)
        onB = big.tile([128, N], f32r)
        nc.gpsimd.memset(onA[64:128, :].bitcast(f32), 0.0)
        nc.gpsimd.memset(onB[64:128, :].bitcast(f32), 0.0)
        nc.gpsimd.memset(onA[64:65, :].bitcast(f32), 1.0)
        den_pad = [big.tile([128, 512], f32r, name="den_pad0"),
                   big.tile([128, 512], f32r, name="den_pad1")]
        nc.gpsimd.memset(den_pad[0][:].bitcast(f32), 0.0)
        nc.gpsimd.memset(den_pad[1][:].bitcast(f32), 0.0)

        # ---------- attention + projection per i-block ----------
        # 36 (jt, h) tiles per i-block; wave sizes alternate 3/2 so the two
        # psum groups (G3: 3 banks, G2: 2 banks) double-buffer; the finished
        # i-block's tail (normalize / project / store) is emitted inside the
        # next i-block's first waves so the PE queue never blocks on it.
        WAVE_SIZES = [3, 3, 2, 3, 2, 3, 2, 3, 2, 3, 2, 3, 2, 3]  # sum = 36

        def make_tail_norm(ib, o, w, av):
            def tail():
                for h, on in ((0, onA), (1, onB)):
                    nc.vector.tensor_copy(den_pad[h][64:65, :w], av[h][64:65, :w])
                    dbc = pav.tile([128, 512], f32, tag="avy", name=f"dbc_{ib}_{h}")
                    nc.tensor.matmul(
                        dbc[:, :w], e64[:], den_pad[h][:, :w],
                        start=True, stop=True,
                    )
                    rb = work.tile([128, 512], f32, tag="rb")
                    nc.vector.reciprocal(rb[:, :w], dbc[:, :w])
                    nc.vector.tensor_tensor(
                        on[0:64, o : o + w], av[h][0:64, :w], rb[0:64, :w], OP.mult
                    )
            return tail

        def make_tail_proj(ib, o, w):
            def tail():
                ysb = ywork.tile([128, KT, 512], f32, tag="y")
                for ot in range(KT):
                    py = pav.tile([128, 512], f32, tag="avy", name=f"py_{ib}_{ot}")
                    nc.tensor.matmul(
                        py[:, :w],
                        wp1_s[:, ot * 128 : (ot + 1) * 128],
                        onA[:, o : o + w],
                        start=True,
                        stop=False,
                    )
                    nc.tensor.matmul(
                        py[:, :w],
                        wp2_s[:, ot * 128 : (ot + 1) * 128],
                        onB[:, o : o + w],
                        start=False,
                        stop=True,
                    )
                    nc.vector.tensor_tensor(
                        ysb[:, ot, :w], py[:, :w], x4s[:, ot, o : o + w], OP.add
                    )
                nc.sync.dma_start(
                    y_d.rearrange("(a p) n -> p a n", p=128)[:, :, o : o + w],
                    ysb[:, :, :w],
                )
            return tail

        deferred = []
        for ib, (o, w) in enumerate(NBLKS):
            av = [
                pav.tile([65, 512], f32, tag="avy", name=f"av0_{ib}"),
                pav.tile([65, 512], f32, tag="avy", name=f"av1_{ib}"),
            ]
            pending = None
            t = 0
            for wv_i, sz in enumerate(WAVE_SIZES):
                tag = "G3" if sz == 3 else "G2"
                g = pg.tile([128, sz, 512], f32, tag=tag, name=f"g_{ib}_{wv_i}")
                tiles = []
                for slot in range(sz):
                    jt, h = t // 2, t % 2
                    t += 1
                    tiles.append((slot, jt, h))
                    nc.tensor.matmul(
                        g[:, slot, :w],
                        k2[64 * h : 64 * (h + 1), jt * 128 : (jt + 1) * 128],
                        q2[64 * h : 64 * (h + 1), o : o + w],
                        start=True,
                        stop=True,
                    )
                p_sb = pwav.tile([128, sz, 512], f32r, tag="P", name=f"p_{ib}_{wv_i}")
                nc.scalar.activation(p_sb[:, :, :w], g[:, :, :w], AF.Exp)
                if debug and ib == 0 and wv_i == 0:
                    nc.sync.dma_start(dbg["P0"], p_sb[:, :, :].bitcast(f32))
                if deferred and wv_i == 0:
                    deferred.pop(0)()
                waves = [pending, (tiles, p_sb)] if pending else [(tiles, p_sb)]
                if wv_i < len(WAVE_SIZES) - 1:
                    pending = waves.pop()
                need_jt = max(
                    (jt for tl, _ in waves for _, jt, _ in tl), default=-1
                )
                while vt_jobs and (JT - len(vt_jobs)) <= need_jt:
                    vt_jobs.pop(0)()
                if vt_jobs:
                    vt_jobs.pop(0)()
                for tiles_j, psb_j in waves:
                    for slot, jt, h in tiles_j:
                        nc.tensor.matmul(
                            av[h][:, :w],
                            vT[:, jt, 65 * h : 65 * h + 65],
                            psb_j[:, slot, :w],
                            start=(jt == 0),
                            stop=(jt == JT - 1),
                            skip_group_check=True,
                        )
            make_tail_norm(ib, o, w, av)()
            deferred = [make_tail_proj(ib, o, w)]
        for fn in deferred:
            fn()

        if debug:
            nc.sync.dma_start(dbg["q2"], q2[:].bitcast(f32))
            nc.sync.dma_start(dbg["k2"], k2[:].bitcast(f32))
            nc.sync.dma_start(dbg["vT"], vT[:].bitcast(f32))
            nc.sync.dma_start(dbg["onA"], onA[:].bitcast(f32))
            nc.sync.dma_start(dbg["onB"], onB[:].bitcast(f32))
            nc.sync.dma_start(dbg["s_bc"], s_bc[:])
            nc.sync.dma_start(dbg["sq_bc"], sq_bc[:])
            nc.sync.dma_start(dbg["s_colT"], s_colT[:])
        for pool in (ywork, pwav, pav, pg, work, big):
            pool.release()

    nc.compile()
    return nc


def _get_program():
    if "nc" not in _CACHE:
        _CACHE["nc"] = _build_program()
    return _CACHE["nc"]


def make_in_maps(x, g, w_qkv, w_out, b_out):
    """Build the per-core input dicts for the SPMD launch."""
    x = np.asarray(x, dtype=np.float32)
    g = np.asarray(g, dtype=np.float32).reshape(DIM)
    w_qkv = np.asarray(w_qkv, dtype=np.float32)
    w_out = np.asarray(w_out, dtype=np.float32)
    b_out = np.asarray(b_out, dtype=np.float32)

    in_maps = []
    for c in range(8):
        beta = c // 4
        h0 = 2 * (c % 4)
        h1 = h0 + 1
        x4 = (x[beta].reshape(DIM, N) / 4.0).astype(np.float32)
        # w_qkv rows: q block [0:512], k block [512:1024], v block [1024:1536]
        qr = np.r_[h0 * DH : (h0 + 1) * DH, h1 * DH : (h1 + 1) * DH]
        wq = w_qkv[qr]            # [128, DIM]
        wk = w_qkv[DIM + qr]      # [128, DIM]
        wvv = w_qkv[2 * DIM + qr]  # [128, DIM]
        gw = (g[None, :] * 4.0).astype(np.float32)
        wqk = np.concatenate([wq * gw, wk * gw], axis=0).T.copy()  # [DIM, 256]
        wvt = (wvv * gw).T.copy()  # [DIM, 128]
        wp1 = np.zeros((128, DIM), dtype=np.float32)
        wp1[0:DH] = w_out[:, h0 * DH : (h0 + 1) * DH].T
        wp1[DH] = b_out / 4.0
        wp2 = np.zeros((128, DIM), dtype=np.float32)
        wp2[0:DH] = w_out[:, h1 * DH : (h1 + 1) * DH].T
        in_maps.append(
            {
                "x4": np.ascontiguousarray(x4),
                "wqk": np.ascontiguousarray(wqk),
                "wv": np.ascontiguousarray(wvt),
                "wp1": wp1,
                "wp2": wp2,
            }
        )
    return in_maps


def run_spmd(in_maps, trace=False):
    from concourse.bass_utils import run_bass_kernel_spmd

    nc = _get_program()
    return run_bass_kernel_spmd(nc, in_maps, list(range(8)), trace=trace)


def combine(results, x):
    x = np.asarray(x, dtype=np.float32)
    y = np.zeros((B, DIM, N), dtype=np.float32)
    for c in range(8):
        y[c // 4] += results[c]["y"]
    return y.reshape(B, DIM, HWS, HWS)


def kernel(x, g, w_qkv, w_out, b_out):
    in_maps = make_in_maps(x, g, w_qkv, w_out, b_out)
    res = run_spmd(in_maps)
    return combine(res.results, x)
